# revision 150
# baseline (speedup 1.0000x reference)
"""TRN2 Bass/Tile kernel: causal self-attention with RoPE.

Sharding across 8 NeuronCores: batch (2) x head-groups (4 groups of 4 heads,
tensor parallel). Each core computes, for its batch and its 4 heads, the
Q/K/V projections, RoPE, causal softmax attention, and a partial output
projection; the host sums the 4 partial outputs per batch.

Key structure (chosen against the TimelineSim cost model, where a matmul
costs out-free-size x pe_cycle regardless of contraction/partition dims):
- All matmul operands bf16 (1 cyc/row; fp8 would blow the accuracy gate).
- RoPE via feature reorder [evens|odds] per head + partition-offset DMA
  swap of the projection tile, then cheap bf16 DVE mults (no doubled
  weight matmuls).
- Scores in [k-part, q-free] orientation; exp on ACT writes only the
  causally valid region; the 128x128 diagonal triangle is masked by one
  small DVE multiply; fully masked q-chunks are never computed or read.
- PV flipped: wt is the stationary operand, V ([s-part, feat]) moving, so
  attention comes out [q-part, feat] with the softmax denominator as a
  per-partition column (ones column in V) -> tensor_scalar normalize.
- PE transpose (bf16) re-orients normalized attention for the output
  projection.
- The issue order software-pipelines PE: within a band, scores(kt+1) is
  issued before PV(kt), and proj/outproj work is spread into the band's
  ACT-bound ktile stream in ~500ns matmul-level slices.
- PSUM->SBUF copies are routed per-phase between DVE and ACT (both engine
  queues are strictly in-order, so a copy queued ahead of an exp or a norm
  stalls the dependent PE work): proj copies ride ACT while it has exp
  slack (st0-2), outproj copies ride DVE except where ACT is free.
- All 16 outproj chunks are issued INLINE into band 3's ktile stream
  (4 mm/fin units after each hp1 ktile) rather than weight-paced: band 3
  is ACT-cadence-bound (~1278ns/ktile incl. exp ack+sem latency) and
  needs ~1.5 outproj ds-units of PE filler per ktile to stay busy.
- Band-3 hp1's per-qc tail uses PE transposes (53ns) + small DVE copies
  for cT instead of ~2.5us DMA transposes; the last two chunks draw
  PSUM from the scores pool (free after the final exp); out-DMAs are
  merged to one [128,1024] per chunk (HWDGE is a single shared device
  at ~625ns per DMA).
"""
import numpy as np
import ml_dtypes
import concourse.bass as bass
from concourse import bacc
import concourse.mybir as mybir
import concourse.tile as tile
from concourse.bass_utils import run_bass_kernel_spmd

B, S, D = 2, 2048, 1024
H, DK = 16, 64
THETA = 10000.0
ST = 512              # q-band / projection s-tile width
NSC = S // 128        # 16 s-chunks of 128
f32 = mybir.dt.float32
bf16 = mybir.dt.bfloat16
AF = mybir.ActivationFunctionType
Alu = mybir.AluOpType

_NC = None
_CONSTS = None


def _interleave(primary, front, even):
    """Issue primary units in order, pacing companion closures against the
    primaries' estimated PE-idle gaps. `front` closures are paced at 2x
    (finish ~halfway through); `even` closures spread across the whole
    stream. All lists hold (weight_ns, fn)."""
    ftot = sum(w for w, _ in front)
    etot = sum(w for w, _ in even)
    gap_total = sum(g for g, _ in primary) or 1.0
    gap_cum = fcum = ecum = 0.0
    fi = ei = 0
    for g, f in primary:
        f()
        gap_cum += g
        frac = gap_cum / gap_total
        while fi < len(front) and fcum < ftot * min(1.0, 1.0 * frac):
            w, cf = front[fi]
            cf()
            fcum += w
            fi += 1
        while ei < len(even) and ecum < etot * frac:
            w, cf = even[ei]
            cf()
            ecum += w
            ei += 1
    for _, cf in front[fi:]:
        cf()
    for _, cf in even[ei:]:
        cf()


def _build():
    import os
    phases = os.environ.get("K_PHASES", "ABC")
    debug = os.environ.get("K_DEBUG", "") == "1"
    nc = bacc.Bacc()
    xT = nc.dram_tensor("xT", [D, S], bf16, kind="ExternalInput")
    pw = nc.dram_tensor("pw", [D, 768], bf16, kind="ExternalInput")
    woT = nc.dram_tensor("woT", [256, D], bf16, kind="ExternalInput")
    cs = nc.dram_tensor("cs", [128, 2, S], bf16, kind="ExternalInput")
    tri = nc.dram_tensor("tri", [128, 256], bf16, kind="ExternalInput")
    eye = nc.dram_tensor("eye", [128, 128], bf16, kind="ExternalInput")
    out = nc.dram_tensor("out", [S, D], bf16, kind="ExternalOutput")
    if debug:
        dbg = {
            nm: nc.dram_tensor(nm, shp, bf16, kind="ExternalOutput")
            for nm, shp in (("d_qT0", [128, S]), ("d_qT1", [128, S]),
                            ("d_kT0", [128, S]), ("d_kT1", [128, S]),
                            ("d_cT0", [128, S]), ("d_cT1", [128, S]),
                            ("d_v", [128, NSC, 4, 65]),
                            ("d_wt0", [128, 1024]), ("d_wt1", [128, 1024]),
                            ("d_wt2", [128, 1024]), ("d_wt3", [128, 1024]),
                            ("d_pvA", [128, 260]), ("d_pvB", [128, 260]))
        }

    with tile.TileContext(nc) as tc:
        with tc.tile_pool(name="persist", bufs=1) as pp, \
             tc.tile_pool(name="pax", bufs=4) as pax, \
             tc.tile_pool(name="scpp", bufs=2, space="PSUM") as scpp, \
             tc.tile_pool(name="misc", bufs=2, space="PSUM") as misc, \
             tc.tile_pool(name="pspv", bufs=1, space="PSUM") as pspv, \
             tc.tile_pool(name="sswp", bufs=3) as sswp, \
             tc.tile_pool(name="st12", bufs=4) as st12, \
             tc.tile_pool(name="wtp", bufs=6) as wtp, \
             tc.tile_pool(name="anp", bufs=5) as anp, \
             tc.tile_pool(name="obp", bufs=6) as obp, \
             tc.tile_pool(name="nrm", bufs=6) as nrm:

            qT = [pp.tile([128, S], bf16, tag=f"qT{i}", name=f"qT{i}")
                  for i in range(2)]
            kT = [pp.tile([128, S], bf16, tag=f"kT{i}", name=f"kT{i}")
                  for i in range(2)]
            cT = [pp.tile([128, S], bf16, tag=f"cT{i}", name=f"cT{i}")
                  for i in range(2)]
            v_sb = pp.tile([128, NSC, 4, 65], bf16, tag="vsb")
            woT_sb = pp.tile([128, 2, D], bf16, tag="woT")
            cs_sb = pp.tile([128, 2, S], bf16, tag="cs")
            pw_sb = pp.tile([128, 8, 768], bf16, tag="pw")
            tri_sb = pp.tile([128, 256], bf16, tag="tri")
            eye_sb = pp.tile([128, 128], bf16, tag="eye")

            pw4 = pw[:].rearrange("(k p) m -> p k m", p=128)
            tri3 = tri_sb[:].rearrange("p (h q) -> p h q", h=2)
            xs_tiles = {}

            def u_xload(st):
                def f():
                    xs = pax.tile([128, 8, ST], bf16, tag="xs", name=f"xs{st}")
                    x4 = xT[:, st * ST:(st + 1) * ST].rearrange(
                        "(k p) m -> p k m", p=128)
                    nc.sync.dma_start(xs[:, 0:4, :], x4[:, 0:4, :])
                    nc.sync.dma_start(xs[:, 4:8, :], x4[:, 4:8, :])
                    xs_tiles[st] = xs
                return (0, f)

            # pw column layout: [qk-hp0 (q128|k128) | qk-hp1 | v 256]
            def projqk_closures(st, hp, act_copy=False):
                sl = slice(st * ST, (st + 1) * ST)
                state = {}
                cls = []

                def mk_mms(half, base, lo):
                    def f():
                        if "qk_sb" not in state:
                            state["qk_sb"] = sswp.tile([128, 1024], bf16,
                                                       tag="qk", name="qksb")
                        if half not in state:
                            state[half] = misc.tile([128, 512], f32, tag="m1",
                                                    name="qkps")
                        h_ps = state[half]
                        xs = xs_tiles[st]
                        for kt in range(lo, lo + 2):
                            nc.tensor.matmul(
                                h_ps[:],
                                pw_sb[:, kt, base:base + 128],
                                xs[:, kt, :],
                                start=(kt == 0), stop=(kt == 7))
                    return f

                def mk_copy(half):
                    # ACT takes the PSUM->SBUF copy when it has slack so the
                    # DVE queue (RoPE chain) doesn't gate PSUM slot recycling
                    def f():
                        dst = state["qk_sb"][:, half * 512:(half + 1) * 512]
                        if act_copy:
                            nc.scalar.copy(dst, state[half][:])
                        else:
                            nc.vector.tensor_copy(dst, state[half][:])
                    return f

                def half_fin(half):
                    # t2 = partner-swapped qk * sign-folded sin, via 4
                    # partition-offset DVE ops (xor-32 block swap)
                    def f():
                        qk_sb = state["qk_sb"]
                        hs = slice(half * 512, (half + 1) * 512)
                        dst = (qT, kT)[half]
                        t1 = st12.tile([128, ST], bf16, tag="t1", name="t1")
                        t2 = st12.tile([128, ST], bf16, tag="t2", name="t2")
                        nc.vector.tensor_tensor(t1[:], qk_sb[:, hs],
                                                cs_sb[:, 0, sl], Alu.mult)
                        for b4 in range(4):
                            o = 32 * b4
                            i = 32 * (b4 ^ 1)
                            # both SB inputs must share base partition: the
                            # sin table carries the sign for the PARTNER row
                            # (S[p] = +sin for even blocks, -sin for odd)
                            nc.vector.tensor_tensor(
                                t2[o:o + 32, :], qk_sb[i:i + 32, hs],
                                cs_sb[i:i + 32, 1, sl], Alu.mult)
                        nc.vector.tensor_tensor(dst[hp][:, sl], t1[:],
                                                t2[:], Alu.add)
                    return f

                for half in range(2):
                    base = 256 * hp + 128 * half
                    for lo in range(0, 8, 2):
                        cls.append((427, mk_mms(half, base, lo)))
                    cls.append((0, mk_copy(half)))
                    cls.append((0, half_fin(half)))
                return cls

            def projv_closures(st, scl, act_copy=False):
                sc = st * 4 + scl
                state = {}

                def mk_mms(lo):
                    def f():
                        if "vp" not in state:
                            state["vp"] = misc.tile([128, 512], f32, tag="m1",
                                                    name="vp")
                        vp = state["vp"]
                        xs = xs_tiles[st]
                        for kt in range(lo, lo + 4):
                            nc.tensor.matmul(
                                vp[:, 0:256],
                                xs[:, kt, scl * 128:(scl + 1) * 128],
                                pw_sb[:, kt, 512:768],
                                start=(kt == 0), stop=(kt == 7))
                    return f

                def fin():
                    src = state["vp"][:, 0:256].rearrange("p (h f) -> p h f",
                                                          h=4)
                    if act_copy:
                        nc.scalar.copy(v_sb[:, sc, :, 0:64], src)
                    else:
                        nc.vector.tensor_copy(v_sb[:, sc, :, 0:64], src)
                return [(427, mk_mms(0)), (427, mk_mms(4)), (0, fin)]

            def dma_closure(*pairs):
                def f():
                    for dst, src in pairs:
                        nc.sync.dma_start(dst, src)
                return (0, f)

            def proj_closures(st):
                cls = projqk_closures(st, 0)
                cls += projqk_closures(st, 1)
                for scl in range(4):
                    cls += projv_closures(st, scl)
                return cls

            def outproj_closures(b, per_chunk=False, act_sel=None,
                                 late_pool_scs=()):
                # per sc: mms ds0, copy ds0, mms ds1, copy ds1 + ONE merged
                # [128, 1024] out-DMA (halves HWDGE pressure). act_sel(sc, ds)
                # routes a copy to ACT (where it has slack) instead of DVE.
                # late_pool_scs chunks draw PSUM from the scores pool (free
                # once the last exp has run) to decouple the band-3 tail from
                # the misc-slot rotation.
                if act_sel is None:
                    act_sel = lambda sc, ds: False
                chunks = []
                for sc in range(b * 4, b * 4 + 4):
                    ssl = slice(sc * 128, (sc + 1) * 128)
                    state = {}

                    def mk_mms(sc, ssl, state, ds):
                        dsl = slice(ds * 512, (ds + 1) * 512)

                        def f():
                            if "ob" not in state:
                                state["ob"] = obp.tile([128, D], bf16,
                                                       tag="ob", name="ob")
                            if sc in late_pool_scs:
                                op_ps = scpp.tile([128, 512], f32, tag="scp",
                                                  name="opps")
                            else:
                                op_ps = misc.tile([128, 512], f32, tag="m1",
                                                  name="opps")
                            state[ds] = op_ps
                            for hp in range(2):
                                nc.tensor.matmul(op_ps[:],
                                                 cT[hp][:, ssl],
                                                 woT_sb[:, hp, dsl],
                                                 start=(hp == 0),
                                                 stop=(hp == 1))
                        return f

                    def mk_fin(sc, ssl, state, ds, use_act, split_dma=False):
                        dsl = slice(ds * 512, (ds + 1) * 512)

                        def f():
                            if use_act:
                                nc.scalar.copy(state["ob"][:, dsl],
                                               state[ds][:])
                            else:
                                nc.vector.tensor_copy(state["ob"][:, dsl],
                                                      state[ds][:])
                            if split_dma:
                                # final chunk: per-half DMAs so the ds0 half
                                # is already in flight during ds1's work and
                                # the very last transfer is half-sized
                                nc.sync.dma_start(out[ssl, dsl],
                                                  state["ob"][:, dsl])
                            elif ds == 1:
                                nc.sync.dma_start(out[ssl, :], state["ob"][:])
                        return f

                    sp = False
                    chunks.append([
                        (427, mk_mms(sc, ssl, state, 0)),
                        (0, mk_fin(sc, ssl, state, 0, act_sel(sc, 0),
                                   split_dma=sp)),
                        (427, mk_mms(sc, ssl, state, 1)),
                        (0, mk_fin(sc, ssl, state, 1, act_sel(sc, 1),
                                   split_dma=sp)),
                    ])
                if per_chunk:
                    return chunks
                return [c for ch in chunks for c in ch]

            def u_ktile(b, hp, kt, pvs):
                def f():
                    nkt = 4 * b + 4
                    j = kt - 4 * b
                    # exp + tri for kt (scores already issued)
                    off = 128 * j if j > 0 else 0
                    scp3 = pvs["scp"][kt][:].rearrange("p (h q) -> p h q", h=2)
                    wt = wtp.tile([128, 1024], bf16, tag="wt", name="wt")
                    wt3 = wt[:].rearrange("p (h q) -> p h q", h=2)
                    if debug and b == 0 and hp == 0:
                        nc.vector.memset(wt[:], 0.0)
                    nc.scalar.activation(wt3[:, :, off:512],
                                         scp3[:, :, off:512],
                                         AF.Exp, scale=0.125)
                    if j >= 0:
                        nc.vector.tensor_tensor(
                            wt3[:, :, 128 * j:128 * j + 128],
                            wt3[:, :, 128 * j:128 * j + 128],
                            tri3[:], Alu.mult)
                    if debug and b == 0 and hp == 0:
                        nc.sync.dma_start(dbg[f"d_wt{kt}"][:], wt[:])
                    # scores for kt+1 ahead of PV(kt)
                    if kt + 1 < nkt:
                        issue_scores(b, hp, kt + 1, pvs)
                    for h in range(2):
                        for qc in range(max(0, j), 4):
                            # start=True marks the WHOLE 2KB psum bank
                            # pending-zero, so only the first matmul into
                            # each pv bank per band may carry it; later
                            # groups' first writes are zeroed by the same
                            # bank-wide flag.
                            nc.tensor.matmul(
                                pvs["pv"][h][:, 65 * qc:65 * qc + 65],
                                wt3[:, h, 128 * qc:128 * qc + 128],
                                v_sb[:, kt, 2 * hp + h, :],
                                start=(kt == 0 and qc == 0),
                                stop=(kt == 4 * b + qc),
                                skip_group_check=True)
                return f

            def issue_scores(b, hp, kt, pvs):
                j = kt - 4 * b
                off = 128 * j if j > 0 else 0
                ktsl = slice(kt * 128, (kt + 1) * 128)
                scp = scpp.tile([128, 1024], f32, tag="scp", name="scp")
                pvs["scp"][kt] = scp
                scp3 = scp[:].rearrange("p (h q) -> p h q", h=2)
                for h in range(2):
                    nc.tensor.matmul(
                        scp3[:, h, off:512],
                        kT[hp][64 * h:64 * h + 64, ktsl],
                        qT[hp][64 * h:64 * h + 64,
                               b * ST + off:(b + 1) * ST],
                        start=True, stop=True)

            def u_norm_dve(b, hp, pvs, state):
                def f():
                    if debug and b == 0 and hp == 0:
                        for h, nm in ((0, "d_pvA"), (1, "d_pvB")):
                            dcp = nrm.tile([128, 260], bf16, tag="dcp",
                                           name="dcp")
                            nc.vector.tensor_copy(dcp[:], pvs["pv"][h][:])
                            nc.sync.dma_start(dbg[nm][:], dcp[:])
                    rcs = []
                    for h in range(2):
                        rc = nrm.tile([128, 4], f32, tag="rc", name="rc")
                        nc.vector.reciprocal_approx_fast(
                            rc[:],
                            pvs["pv"][h][:].rearrange("p (qc e) -> p qc e",
                                                      e=65)[:, :, 64])
                        rcs.append(rc)
                    an4 = anp.tile([128, 512], bf16, tag="an", name="an")
                    for qc in range(4):
                        for h in range(2):
                            nc.vector.tensor_scalar(
                                an4[:, 128 * qc + 64 * h:
                                    128 * qc + 64 * h + 64],
                                pvs["pv"][h][:, 65 * qc:65 * qc + 64],
                                rcs[h][:, qc:qc + 1], None, Alu.mult)
                    def dma():
                        nc.sync.dma_start(
                            cT[hp][:, b * ST:(b + 1) * ST]
                            .rearrange("p (qc q) -> p qc q", qc=4),
                            an4[:], transpose=True)
                    state["dma"] = dma
                    state["an4"] = an4
                return f

            def u_tp_band(b, hp, qc, state):
                # PE-transpose one 128q x 128f block of the band's normalized
                # attention into cT (replaces the ~2.5us DMA transpose on the
                # band-3 hp0 path, which gates the whole outproj tail)
                def f():
                    tp = misc.tile([128, 128], bf16, tag="m1", name="tpb")
                    nc.tensor.transpose(
                        tp[:], state["an4"][:, 128 * qc:128 * qc + 128],
                        eye_sb[:])
                    nc.vector.tensor_copy(
                        cT[hp][:, b * ST + qc * 128:b * ST + qc * 128 + 128],
                        tp[:])
                return f

            def u_norm_qc(b, hp, pvs, qc, state):
                # per-q-chunk normalize into `an` (band tail); transpose is a
                # separate PE unit (u_tp_qc) to keep DMA latency off the tail
                def f():
                    rcs = []
                    for h in range(2):
                        rc = nrm.tile([128, 4], f32, tag="rc", name="rc")
                        nc.vector.reciprocal_approx_fast(
                            rc[:, 0:1],
                            pvs["pv"][h][:, 65 * qc + 64:65 * qc + 65])
                        rcs.append(rc)
                    an = anp.tile([128, 512], bf16, tag="an", name="an")
                    for h in range(2):
                        nc.vector.tensor_scalar(
                            an[:, 64 * h:64 * h + 64],
                            pvs["pv"][h][:, 65 * qc:65 * qc + 64],
                            rcs[h][:, 0:1], None, Alu.mult)
                    state["an"] = an
                return f

            def u_tp_qc(b, hp, qc, state, use_act=False):
                # PE-transpose the normalized [128q, 128f] block into cT and
                # copy PSUM->SBUF; ~350ns vs ~2.5us for a DMA transpose
                def f():
                    tp = misc.tile([128, 128], bf16, tag="m1", name="tp")
                    nc.tensor.transpose(tp[:], state["an"][:, 0:128],
                                        eye_sb[:])
                    dst = cT[hp][:, b * ST + qc * 128:b * ST + qc * 128 + 128]
                    if use_act:
                        nc.scalar.copy(dst, tp[:])
                    else:
                        nc.vector.tensor_copy(dst, tp[:])
                return f

            def band_units(b, tail_cls=(), inline_cls=None, inline_rate=(0, 4)):
                # returns [(gap_ns, fn)] with per-unit PE-idle estimates.
                # inline_cls units are attached directly after ktile units
                # (inline_rate[hp] per ktile) so late-band filler lands in the
                # hp1 stream instead of being consumed early by hp0.
                inline_iter = iter(inline_cls) if inline_cls else None
                inline_taper = ([0] * 16,
                [5] * 4 + [4] * 6 + [2] * 2 + [0] * 4)
                us = []
                nkt = 4 * b + 4
                for hp in range(2):
                    pvA = pspv.tile([128, 260], f32, tag="pvA",
                                    name=f"pvA{b}{hp}")
                    pvB = pspv.tile([128, 260], f32, tag="pvB",
                                    name=f"pvB{b}{hp}")
                    pvs = {"pv": (pvA, pvB), "scp": {}}

                    def mk_start(b, hp, pvs):
                        def f():
                            issue_scores(b, hp, 0, pvs)
                        return f
                    us.append((100 if hp == 0 else 0, mk_start(b, hp, pvs)))
                    if b == 3 and hp == 0:
                        for scl in range(4):
                            for w_, fn_ in projv_closures(3, scl):
                                us.append((0, fn_))
                    vp_embed = {}
                    for kt in range(nkt):
                        for w_, fn_ in vp_embed.get(kt, ()):
                            us.append((0, fn_))
                        j = kt - 4 * b
                        off = 128 * j if j > 0 else 0
                        act = (1024 - 2 * off) * 0.833 + 185
                        nxt = kt + 1
                        pe = 0.0
                        if nxt < nkt:
                            offn = 128 * (nxt - 4 * b) if nxt > 4 * b else 0
                            pe += (1024 - 2 * offn) * 0.4167
                        pe += (8 - 2 * max(0, j)) * 27.1
                        us.append((max(0.0, act - pe),
                                   u_ktile(b, hp, kt, pvs)))
                        if inline_iter is not None:
                            n_in = inline_taper[hp][kt]
                            for _ in range(n_in):
                                nxt = next(inline_iter, None)
                                if nxt is not None:
                                    us.append((0, nxt[1]))
                    if b == 3 and hp == 1 and tail_cls:
                        # pipelined tail: per-qc norm + PE-transpose + outproj
                        # interleaved into the diag ktiles (qc's pv completes
                        # at kt=12+qc). Chunk qc-1's copies are issued AFTER
                        # ktile 12+qc so they queue behind exp(12+qc) on the
                        # in-order ACT/DVE queues, not ahead of it.
                        norms = []
                        for qc in range(4):
                            nst = {}
                            norms.append([
                                (500, u_norm_qc(b, hp, pvs, qc, nst)),
                                (0, u_tp_qc(b, hp, qc, nst))])
                        base = len(us) - 4  # kt12..15 units at base..base+3
                        rebuilt = us[:base]
                        for qc in range(4):
                            rebuilt.append(us[base + qc])
                            if qc in (2, 3):
                                # chunk qc-2's ds1 fin deferred past norm qc
                                # so its DVE copy can't queue ahead of it
                                rebuilt.append((0, tail_cls[qc - 2][3][1]))
                            if qc > 0:
                                # chunk qc-1's ds0 mms fill PE while the
                                # norm->transpose DVE chain for qc runs
                                rebuilt.append((0, tail_cls[qc - 1][0][1]))
                            rebuilt.extend(norms[qc])
                            if qc > 0:
                                rebuilt.extend(
                                    (0, fn_)
                                    for _w, fn_ in tail_cls[qc - 1][1:3])
                        rebuilt.append((0, tail_cls[2][3][1]))
                        rebuilt.extend((0, fn_) for _w, fn_ in tail_cls[3])
                        us = rebuilt
                    else:
                        nstate = {}
                        us.append((850, u_norm_dve(b, hp, pvs, nstate)))
                        us.append((200, lambda nstate=nstate: nstate["dma"]()))
                if inline_iter is not None:
                    # flush any inline units the per-ktile rate didn't place
                    for nxt in inline_iter:
                        us.append((0, nxt[1]))
                return us

            # ---- pipeline schedule ----
            # 256-col pw pieces keep DMA elem size >= 512B (no 2x descriptor
            # penalty); xs/pw alternate so the PE mm stream starts ~3.3us and
            # stays just behind the DMA stream.
            xs0 = pax.tile([128, 8, ST], bf16, tag="xs", name="xs0")
            x40 = xT[:, 0:ST].rearrange("(k p) m -> p k m", p=128)
            nc.sync.dma_start(xs0[:, 0:2, :], x40[:, 0:2, :])
            nc.scalar.dma_start(pw_sb[:, 0:2, 0:256], pw4[:, 0:2, 0:256])
            nc.sync.dma_start(xs0[:, 2:4, :], x40[:, 2:4, :])
            nc.scalar.dma_start(pw_sb[:, 2:8, 0:256], pw4[:, 2:8, 0:256])
            nc.sync.dma_start(xs0[:, 4:6, :], x40[:, 4:6, :])
            nc.sync.dma_start(xs0[:, 6:8, :], x40[:, 6:8, :])
            xs_tiles[0] = xs0
            nc.sync.dma_start(cs_sb[:, :, 0:512], cs[:, :, 0:512])
            nc.scalar.dma_start(pw_sb[:, :, 256:512], pw4[:, :, 256:512])
            nc.scalar.dma_start(pw_sb[:, :, 512:768], pw4[:, :, 512:768])
            nc.sync.dma_start(tri_sb[:], tri[:])
            nc.sync.dma_start(cs_sb[:, :, 512:2048], cs[:, :, 512:2048])
            nc.vector.memset(v_sb[:, :, :, 64], 1.0)
            for _w, f in (projqk_closures(0, 0, act_copy=True)
                          + projqk_closures(0, 1, act_copy=True)):
                f()
            for scl in range(4):
                for _w, f in projv_closures(0, scl, act_copy=True):
                    f()
            u_xload(1)[1]()
            for scl in range(4):
                for _w, f in projv_closures(1, scl, act_copy=True):
                    f()
            u_xload(2)[1]()
            for scl in range(4):
                for _w, f in projv_closures(2, scl, act_copy=True):
                    f()
            if "B" in phases:
                for b in range(4):
                    if 1 < b < 3:
                        u_xload(b + 1)[1]()
                    front = []
                    even = []
                    if b < 3:
                        ac = b < 2  # ACT has copy slack in bands 0-1
                        front += projqk_closures(b + 1, 0, act_copy=ac)
                        front += projqk_closures(b + 1, 1, act_copy=ac)

                    if b == 2:
                        even += [dma_closure(
                            (woT_sb[:],
                             woT[:].rearrange("(k p) m -> p k m", p=128)),
                            (eye_sb[:], eye[:]))]
                    tail = ()
                    inline = None
                    if b == 3 and "C" in phases:
                        inline = (outproj_closures(0) + outproj_closures(1)
                                  + outproj_closures(2))
                        tail = outproj_closures(
                            3, per_chunk=True,
                            act_sel=lambda sc, ds: sc >= 12 and (ds == 0 or sc == 12),
                            late_pool_scs=(14, 15))
                    _interleave(band_units(b, tail, inline_cls=inline),
                                front, even)
            if debug:
                for nm, t in (("d_qT0", qT[0]), ("d_qT1", qT[1]),
                              ("d_kT0", kT[0]), ("d_kT1", kT[1]),
                              ("d_cT0", cT[0]), ("d_cT1", cT[1]),
                              ("d_v", v_sb)):
                    nc.sync.dma_start(dbg[nm][:], t[:])
    nc.finalize()
    return nc


def _host_consts():
    # RoPE tables in [evens|odds] per-32-block row layout, sign folded into
    # the sin table. Row p: freq index p%32; blocks 0,2 (even slots) carry
    # -sin, blocks 1,3 (odd slots) +sin.
    inv_freq = 1.0 / (THETA ** (np.arange(0, DK, 2, dtype=np.float64) / DK))

    tri_np = np.zeros((128, 256), np.float32)
    p = np.arange(128)
    q = np.arange(128)
    blk = (q[None, :] >= p[:, None]).astype(np.float32)
    tri_np[:, 0:128] = blk
    tri_np[:, 128:256] = blk
    eye_np = np.eye(128, dtype=np.float32)
    return (inv_freq,
            tri_np.astype(ml_dtypes.bfloat16),
            eye_np.astype(ml_dtypes.bfloat16))


def _cs_table(pos, inv_freq):
    # pos: [S] int positions for this batch -> cs [128, 2, S] bfloat16
    i = np.arange(128) % 32
    ang = pos[None, :].astype(np.float64) * inv_freq[i][:, None]  # [128, S]
    cs = np.empty((128, 2, len(pos)), np.float32)
    cs[:, 0, :] = np.cos(ang)
    sgn = np.where(((np.arange(128) // 32) % 2) == 0, 1.0, -1.0)
    cs[:, 1, :] = sgn[:, None] * np.sin(ang)
    return cs.astype(ml_dtypes.bfloat16)


_EO_PERM = None


def _eo_perm():
    # per-head [evens | odds] feature permutation for 256 q/k rows (4 heads)
    global _EO_PERM
    if _EO_PERM is None:
        perm = []
        for h in range(4):
            perm.extend(64 * h + np.arange(0, 64, 2))
            perm.extend(64 * h + np.arange(1, 64, 2))
        _EO_PERM = np.asarray(perm)
    return _EO_PERM


def kernel(x, token_positions, W_q, W_k, W_v, W_o):
    global _NC, _CONSTS
    if _NC is None:
        _NC = _build()
    if _CONSTS is None:
        _CONSTS = _host_consts()
    inv_freq, tri_np, eye_np = _CONSTS

    x = np.asarray(x, dtype=np.float32)
    token_positions = np.asarray(token_positions)
    W_q = np.asarray(W_q, dtype=np.float32)
    W_k = np.asarray(W_k, dtype=np.float32)
    W_v = np.asarray(W_v, dtype=np.float32)
    W_o = np.asarray(W_o, dtype=np.float32)

    perm = _eo_perm()
    cs_by_batch = [
        _cs_table(np.asarray(token_positions[b], dtype=np.int64), inv_freq)
        for b in range(B)
    ]
    xT_by_batch = [
        np.ascontiguousarray(x[b].T).astype(ml_dtypes.bfloat16)
        for b in range(B)
    ]

    in_maps = []
    for c in range(8):
        b, g = divmod(c, 4)
        rows = slice(256 * g, 256 * (g + 1))
        wq = W_q[rows][perm]
        wk = W_k[rows][perm]
        wv = W_v[rows]
        # pw columns: [q-hp0 | k-hp0 | q-hp1 | k-hp1 | v]
        pw_np = np.ascontiguousarray(np.concatenate(
            [wq[0:128].T, wk[0:128].T, wq[128:256].T, wk[128:256].T, wv.T],
            axis=1)).astype(ml_dtypes.bfloat16)
        woT_np = np.ascontiguousarray(W_o[:, rows].T).astype(
            ml_dtypes.bfloat16)
        in_maps.append({
            "xT": xT_by_batch[b], "pw": pw_np, "woT": woT_np,
            "cs": cs_by_batch[b], "tri": tri_np, "eye": eye_np,
        })

    res = run_bass_kernel_spmd(_NC, in_maps, core_ids=list(range(8)))
    outs = [np.asarray(res.results[c]["out"], np.float32) for c in range(8)]
    o0 = outs[0] + outs[1] + outs[2] + outs[3]
    o1 = outs[4] + outs[5] + outs[6] + outs[7]
    return np.stack([o0, o1]).astype(np.float32)

